# revision 29
# baseline (speedup 1.0000x reference)
"""Batched GCN (microtubule dynamics model) on 8 Trainium2 NeuronCores.

Math: the reference's gather/scale/scatter message passing over a fixed
52-node graph is a dense linear operator on the node axis:
    agg[b] = A @ h[b],  A[i, j] = sum over edges (j->i, incl self-loops)
                                   of dinv[src] * dinv[dst]
and A commutes with the shared linear layer, so each GNN layer is
    x += relu((A @ x) @ W_l^T + b_l),  batched over B.

Device strategy (pure data parallel, 512 batch elems / core):
  - activations live in SBUF as [128 hid partitions, 26624 token cols]
    (token = b*52 + node), fp16 on-chip, fp32 PSUM for the W-matmuls
  - per layer, per 16-batch-elem group (832 cols):
      PE-transpose 8 pairs of batch elems (104 tokens each) straight out
      of X into node-on-partition layout -> copy to SBUF -> node-mix as
      stationary-side matmuls (lhsT = transposed x chunk, rhs =
      blockdiag(A^T,A^T)) producing A@x back in hid-major layout -> copy
      -> W-matmul -> fused relu+bias (ACT) -> residual add (DVE)
  - encoder: [6, TOK] feature-major input prepared on host; relu+bias
    alternates between ACT and DVE to balance engines
  - decoder: bias of the final 6-dim layer added on the PE via a rank-1
    accumulating matmul against a ones-row; result DMA'd PSUM->HBM
    directly, so the output path costs no vector-engine time
"""

import numpy as np

N_FIL, N_SUB = 13, 4
N_NODES = N_FIL * N_SUB          # 52
FEAT = 6
HID = 128
N_LAYERS = 3
BATCH = 4096
N_CORES = 8
B_C = BATCH // N_CORES           # 512 batch elems per core
TOK = B_C * N_NODES              # 26624 token columns per core
PAIR_T = 2 * N_NODES             # 104 tokens per transpose chunk
GROUP_B = 8                      # batch elems per inner tile
GROUP_T = GROUP_B * N_NODES      # 832 token columns per inner tile
N_GROUPS = B_C // GROUP_B        # 32
N_PAIRS = GROUP_B // 2           # 8 pairs per group
SLICE = 512                      # encoder/decoder column slice
N_SLICES = TOK // SLICE          # 52

_CACHE = {}
_LAST_EXEC_NS = None
_LAST_TRACE = []


def _build_nc():
    import concourse.bacc as bacc
    import concourse.mybir as mybir
    from concourse.tile import TileContext
    from concourse.masks import make_identity

    f32 = mybir.dt.float32
    f16 = mybir.dt.float16
    Alu = mybir.AluOpType

    nc = bacc.Bacc(trn_type="TRN2")

    qT_d = nc.dram_tensor("qT", [FEAT, TOK], f16, kind="ExternalInput")
    winT_d = nc.dram_tensor("winT", [FEAT, HID], f16, kind="ExternalInput")
    bin_d = nc.dram_tensor("bin", [HID, 1], f32, kind="ExternalInput")
    wgT_d = nc.dram_tensor("wgT", [HID, N_LAYERS, HID], f16, kind="ExternalInput")
    bg_d = nc.dram_tensor("bg", [HID, N_LAYERS], f32, kind="ExternalInput")
    wd1T_d = nc.dram_tensor("wd1T", [HID, HID], f16, kind="ExternalInput")
    bd1_d = nc.dram_tensor("bd1", [HID, 1], f32, kind="ExternalInput")
    wd2T_d = nc.dram_tensor("wd2T", [HID, FEAT], f16, kind="ExternalInput")
    bd2r_d = nc.dram_tensor("bd2r", [1, FEAT], f16, kind="ExternalInput")
    a2_d = nc.dram_tensor("a2", [PAIR_T, PAIR_T], f16, kind="ExternalInput")
    yT_d = nc.dram_tensor("yT", [FEAT, TOK], f16, kind="ExternalOutput")

    Relu = mybir.ActivationFunctionType.Relu

    with TileContext(nc) as tc:
        with (
            tc.sbuf_pool(name="const", bufs=1) as cp,
            tc.sbuf_pool(name="work", bufs=4) as wp,
            tc.psum_pool(name="ps", bufs=2) as pp,
        ):
            ident = cp.tile([128, 128], f16)
            make_identity(nc, ident)
            ones_s = cp.tile([1, SLICE], f16)
            nc.vector.memset(ones_s, 1.0)
            zero_s = cp.tile([128, SLICE], f16)
            nc.vector.memset(zero_s, 0.0)

            winT = cp.tile_from(winT_d[:, :])
            bin_s = cp.tile_from(bin_d[:, :])
            wgT = cp.tile_from(wgT_d[:, :, :])
            bg_s = cp.tile_from(bg_d[:, :])
            wd1T = cp.tile_from(wd1T_d[:, :])
            bd1_s = cp.tile_from(bd1_d[:, :])
            wd2T = cp.tile_from(wd2T_d[:, :])
            bd2r = cp.tile_from(bd2r_d[:, :])
            a2 = cp.tile_from(a2_d[:, :])

            qT = cp.tile([FEAT, TOK], f16)
            nc.sync.dma_start(qT, qT_d[:, :])

            X = cp.tile([128, TOK], f16)

            # -------- encoder: X = relu(W_in @ q^T + b_in) ----------------
            for s in range(N_SLICES):
                cols = slice(s * SLICE, (s + 1) * SLICE)
                enc_ps = pp.tile([128, SLICE], f32, tag="ps_c", bufs=4)
                nc.tensor.matmul(
                    enc_ps, winT, qT[:, cols], start=True, stop=True
                )
                if s % 2 == 0:
                    nc.scalar.activation(X[:, cols], enc_ps, Relu, bias=bin_s)
                else:
                    nc.vector.scalar_tensor_tensor(
                        X[:, cols], enc_ps, bin_s, zero_s[:, :SLICE],
                        op0=Alu.add, op1=Alu.max,
                    )

            # -------- GNN layers: x += relu(A (x W_l^T) + b_l) -----------
            # Fused transpose+W-matmul: the pair chunk of X is the
            # stationary operand (as in a PE transpose), but the moving
            # operand is W_l^T instead of the identity, so one matmul
            # yields h^T = (x W_l^T)^T in node-on-partition layout.
            for l in range(N_LAYERS):
                for g in range(N_GROUPS):
                    c0 = g * GROUP_T
                    cols = slice(c0, c0 + GROUP_T)

                    ht_ps = pp.tile(
                        [PAIR_T, 128 * N_PAIRS], f32, tag="ps_b", bufs=4
                    )
                    for p in range(N_PAIRS):
                        nc.tensor.matmul(
                            ht_ps[:, p * 128:(p + 1) * 128],
                            X[:, c0 + p * PAIR_T:c0 + (p + 1) * PAIR_T],
                            wgT[:, l, :],
                            start=True, stop=True,
                        )
                    hts = wp.tile([PAIR_T, 128 * N_PAIRS], f16)
                    if g % 2 == 0:
                        nc.vector.tensor_copy(hts, ht_ps)
                    else:
                        nc.scalar.copy(hts, ht_ps)

                    # node mix back to hid-major: agg[hid,(g,i)] =
                    #   sum_j h^T[(g,j), hid] * A[i,j]
                    agg_ps = pp.tile([128, GROUP_T], f32, tag="ps_c", bufs=4)
                    for p in range(N_PAIRS):
                        nc.tensor.matmul(
                            agg_ps[:, p * PAIR_T:(p + 1) * PAIR_T],
                            hts[:, p * 128:(p + 1) * 128],
                            a2,
                            start=True, stop=True,
                        )

                    # x += relu(agg + b_l)
                    r = wp.tile([128, GROUP_T], f16)
                    if g % 8 < 5:
                        nc.scalar.activation(
                            r, agg_ps, Relu, bias=bg_s[:, l:l + 1]
                        )
                    else:
                        nc.vector.scalar_tensor_tensor(
                            r, agg_ps, bg_s[:, l:l + 1],
                            zero_s[:, :GROUP_T] if GROUP_T <= SLICE else zero_s,
                            op0=Alu.add, op1=Alu.max,
                        )
                    if g % 8 < 5:
                        nc.gpsimd.tensor_add(X[:, cols], X[:, cols], r)
                    else:
                        nc.vector.tensor_add(X[:, cols], X[:, cols], r)

            # -------- decoder --------------------------------------------
            for s4 in range(N_SLICES // 4):
                y4_ps = pp.tile([102, SLICE], f32, tag="ps_b", bufs=4)
                for k in range(4):
                    s = s4 * 4 + k
                    cols = slice(s * SLICE, (s + 1) * SLICE)
                    d1_ps = pp.tile([128, SLICE], f32, tag="ps_c", bufs=4)
                    nc.tensor.matmul(
                        d1_ps, wd1T, X[:, cols], start=True, stop=True
                    )
                    d1s = wp.tile([128, SLICE], f16)
                    if s % 2 == 0:
                        nc.vector.scalar_tensor_tensor(
                            d1s, d1_ps, bd1_s, zero_s[:, :SLICE],
                            op0=Alu.add, op1=Alu.max,
                        )
                    else:
                        nc.scalar.activation(d1s, d1_ps, Relu, bias=bd1_s)

                    # y = W_d2 @ d1 + b_d2, col-tiled to partitions 32k..32k+5
                    nc.tensor.matmul(
                        y4_ps[32 * k:32 * k + FEAT, :], wd2T, d1s,
                        start=True, stop=False, tile_position=(0, 32 * k),
                    )
                    nc.tensor.matmul(
                        y4_ps[32 * k:32 * k + FEAT, :], bd2r, ones_s,
                        start=False, stop=True, tile_position=(0, 32 * k),
                    )
                y4s = wp.tile([102, SLICE], f16)
                if s4 % 2 == 0:
                    nc.vector.tensor_copy(y4s, y4_ps)
                else:
                    nc.scalar.copy(y4s, y4_ps)
                for k in range(4):
                    s = s4 * 4 + k
                    cols = slice(s * SLICE, (s + 1) * SLICE)
                    nc.sync.dma_start(
                        yT_d[:, cols], y4s[32 * k:32 * k + FEAT, :]
                    )

    nc.finalize()
    return nc


def _host_prep(inputs):
    q = np.asarray(inputs["q_current"], np.float32).reshape(BATCH, N_NODES, FEAT)
    W_in = np.asarray(inputs["W_in"], np.float32)
    b_in = np.asarray(inputs["b_in"], np.float32)
    W_gnn = np.asarray(inputs["W_gnn"], np.float32)
    b_gnn = np.asarray(inputs["b_gnn"], np.float32)
    W_d1 = np.asarray(inputs["W_d1"], np.float32)
    b_d1 = np.asarray(inputs["b_d1"], np.float32)
    W_d2 = np.asarray(inputs["W_d2"], np.float32)
    b_d2 = np.asarray(inputs["b_d2"], np.float32)
    edge = np.asarray(inputs["edge_index"]).astype(np.int64)

    # dense normalized adjacency (PyG GCNConv default w/ self-loops)
    loops = np.arange(N_NODES, dtype=np.int64)
    src = np.concatenate([edge[0], loops])
    dst = np.concatenate([edge[1], loops])
    deg = np.zeros(N_NODES, np.float32)
    np.add.at(deg, dst, 1.0)
    dinv = 1.0 / np.sqrt(np.maximum(deg, 1e-12))
    A = np.zeros((N_NODES, N_NODES), np.float32)
    np.add.at(A, (dst, src), dinv[src] * dinv[dst])

    a2 = np.zeros((PAIR_T, PAIR_T), np.float32)
    a2[:N_NODES, :N_NODES] = A.T
    a2[N_NODES:, N_NODES:] = A.T

    const = {
        "winT": np.ascontiguousarray(W_in.T).astype(np.float16),
        "bin": np.ascontiguousarray(b_in.reshape(HID, 1)),
        "wgT": np.ascontiguousarray(W_gnn.transpose(2, 0, 1)).astype(np.float16),
        "bg": np.ascontiguousarray(b_gnn.T),
        "wd1T": np.ascontiguousarray(W_d1.T).astype(np.float16),
        "bd1": np.ascontiguousarray(b_d1.reshape(HID, 1)),
        "wd2T": np.ascontiguousarray(W_d2.T).astype(np.float16),
        "bd2r": np.ascontiguousarray(b_d2.reshape(1, FEAT)).astype(np.float16),
        "a2": a2.astype(np.float16),
    }

    # per-core feature-major input [6, TOK], fp16
    q_flat = q.reshape(N_CORES, B_C * N_NODES, FEAT)
    qTs = [
        np.ascontiguousarray(q_flat[c].T).astype(np.float16)
        for c in range(N_CORES)
    ]
    return const, qTs


def kernel(**inputs):
    const, qTs = _host_prep(inputs)

    if "nc" not in _CACHE:
        _CACHE["nc"] = _build_nc()
    nc = _CACHE["nc"]

    from concourse.bass_utils import run_bass_kernel_spmd

    in_maps = [dict(const, qT=qTs[c]) for c in range(N_CORES)]
    res = run_bass_kernel_spmd(nc, in_maps, core_ids=list(range(N_CORES)))
    global _LAST_EXEC_NS
    _LAST_EXEC_NS = res.exec_time_ns
    if res.instructions_and_trace is not None:
        _LAST_TRACE.append(res.instructions_and_trace[1])

    outs = []
    for c in range(N_CORES):
        yT = res.results[c]["yT"]  # [6, TOK] fp32
        outs.append(np.asarray(yT, np.float32).T)
    y = np.concatenate(outs, axis=0)  # [BATCH*52, 6]
    return np.ascontiguousarray(y).reshape(BATCH, N_FIL, N_SUB, FEAT)
